# revision 1
# baseline (speedup 1.0000x reference)
"""Self-contained Trainium2 (Bass/Tile) kernel for nn_CoreRNNFW_65463891525848.

Strategy
--------
Pure data parallel: batch B=16 is sharded 2-per-core across 8 NeuronCores;
weights are replicated.  Each core runs the full T*S recurrence for its two
samples inside ONE fully-unrolled Bass program.

Layout ("d-layout"): every length-768 vector lives as [128 partitions, 6
chunks], with the per-core batch (2) as an extra free axis -> elementwise ops
are [128, 12] (overhead-bound, ~170ns each) instead of [2, 768].

The fast-weight matrix A_t = sum_s lam^(t-s) * eta * v_s v_s^T / n_s is never
materialized: hist row s stores w_s = v_s * sqrt(eta * lam^-s / n_s), so
A_{t-1} h = lam^(t-1) * sum_s (w_s . h) w_s  -- two small contractions per
inner step. Cross-partition reductions use a ones[128,128] matmul on the PE,
which also broadcasts the result to all partitions (so later per-sample
scalars are consumed as [128, 2] tensors with 0-stride free broadcast APs).

A numpy fallback keeps kernel() correct even if the device path fails.
"""

import math
import numpy as np

T, B = 24, 16
NCORES = 8
BL = B // NCORES          # 2 samples per core
D_G, D_H, D_OUT = 256, 768, 256
KI = D_H // 128           # 6 chunks of the hidden dim
S_IN = 4
LAM = 0.95
ETA = 0.5
EPS = 1e-6
LN_EPS = 1e-5
RANK = T - 1              # 23 hebbian updates


def _softplus(x):
    return np.log1p(np.exp(-abs(float(x)))) + max(float(x), 0.0)


def _compute_k(alpha):
    a = float(alpha)
    if a >= 0:
        return np.float32(1.0 + _softplus(a))
    return np.float32(1.0 / (1.0 + _softplus(-a)))


# ----------------------------------------------------------------------------
# Bass program
# ----------------------------------------------------------------------------

_CACHE = {}


def _build():
    import concourse.bass as bass
    import concourse.tile as tile
    from concourse import bacc, mybir
    from concourse.masks import make_identity

    # All our ACT functions (Ln/Exp/Copy/Relu/Square) live together in the
    # natural_log_exp_and_others table set.  bacc's greedy set chooser
    # otherwise ping-pongs between sets, emitting a ~1.3us ACT_TABLE_LOAD
    # before nearly every activation.  Restrict every other set to the
    # functions set 6 does NOT provide, so one table load serves the kernel.
    if not getattr(bacc, "_act_tables_pinned", False):
        _orig_get_tables = bacc.get_activation_tables

        def _pinned_tables(arch):
            tabs = _orig_get_tables(arch)
            anchor = tabs.get("natural_log_exp_and_others")
            if anchor:
                for name in tabs:
                    if name != "natural_log_exp_and_others":
                        tabs[name] = tabs[name] - anchor
            return tabs

        bacc.get_activation_tables = _pinned_tables
        bacc._act_tables_pinned = True

    fp32 = mybir.dt.float32
    bf16 = mybir.dt.bfloat16
    AX = mybir.AxisListType
    OP = mybir.AluOpType
    AF = mybir.ActivationFunctionType

    def mkap(base, pairs):
        return bass.AP(tensor=base.tensor, offset=base.offset,
                       ap=[list(p) for p in pairs])

    def bcast_ax(ap_obj, pos, count):
        """Insert a 0-stride (broadcast) axis at `pos` (0 = partition dim)."""
        pairs = [list(p) for p in ap_obj.ap]
        pairs.insert(pos, [0, count])
        return bass.AP(tensor=ap_obj.tensor, offset=ap_obj.offset, ap=pairs)

    nc = bacc.Bacc("TRN2", target_bir_lowering=False, debug=False)

    z_d = nc.declare_dram_parameter("z", [T * BL, D_G], fp32, isOutput=False)
    cl_d = nc.declare_dram_parameter("clean", [BL, D_OUT], fp32, isOutput=False)
    wh_d = nc.declare_dram_parameter("W_h", [D_H, D_H], fp32, isOutput=False)
    wg_d = nc.declare_dram_parameter("W_g", [D_H, D_G], fp32, isOutput=False)
    bh_d = nc.declare_dram_parameter("b_h", [D_H], fp32, isOutput=False)
    gm_d = nc.declare_dram_parameter("ln_gamma", [D_H], fp32, isOutput=False)
    bt_d = nc.declare_dram_parameter("ln_beta", [D_H], fp32, isOutput=False)
    hw_d = nc.declare_dram_parameter("head_W", [D_OUT, D_H], fp32, isOutput=False)
    hb_d = nc.declare_dram_parameter("head_b", [D_OUT], fp32, isOutput=False)
    k_d = nc.declare_dram_parameter("k_fw", [1], fp32, isOutput=False)
    loss_d = nc.declare_dram_parameter("loss", [BL, 1], fp32, isOutput=True)

    with tile.TileContext(nc) as tc:
        from contextlib import ExitStack
        with ExitStack() as ctx:
            persist = ctx.enter_context(tc.tile_pool(name="persist", bufs=1))
            work = ctx.enter_context(tc.tile_pool(name="work", bufs=3))

            # ---------------- persistent state ----------------
            ones = persist.tile([128, 128], fp32, tag="ones")
            ones_bf = persist.tile([128, 128], bf16, tag="ones_bf")
            lam_ones = persist.tile([128, 128], bf16, tag="lam_ones")
            id128 = persist.tile([128, 128], fp32, tag="id128")
            idbf = persist.tile([128, 128], bf16, tag="idbf")
            WhT = persist.tile([128, KI, D_H], bf16, tag="WhT")
            hs_bf = persist.tile([128, BL, KI], bf16, tag="hs_bf")
            hWT = persist.tile([128, KI, D_OUT], fp32, tag="hWT")
            Z = persist.tile([128, KI, T * BL], fp32, tag="Z")
            gam = persist.tile([128, KI], fp32, tag="gam")
            bet = persist.tile([128, KI], fp32, tag="bet")
            kbc = persist.tile([128, 1], fp32, tag="kbc")
            hist = persist.tile([128, RANK, BL, KI], fp32, tag="hist")
            S = persist.tile([128, 2, BL, KI], fp32, tag="S")      # [hs | Ah]
            hs2 = persist.tile([128, 2, BL, KI], fp32, tag="hs2")  # [h' | h'^2]
            hbase = persist.tile([128, BL, KI], fp32, tag="hbase")
            hbT = persist.tile([BL, D_H], bf16, tag="hbT")
            hc = persist.tile([BL, D_OUT], fp32, tag="hc")      # clean - head_b
            nrm = persist.tile([BL, 1], fp32, tag="nrm")
            dump = persist.tile([128, 1], fp32, tag="dump")
            lneps_t = persist.tile([128, 1], fp32, tag="lneps")
            eps_t = persist.tile([128, 1], fp32, tag="epst")
            mln_t = persist.tile([128, 1], fp32, tag="mlnt")

            cs12 = persist.tile([128, 4], fp32, tag="cs12")
            nc.vector.memset(cs12[:, 0:2], 1.0)
            nc.vector.memset(cs12[:, 2:4], 2.0)
            nc.vector.memset(lneps_t[:, :], LN_EPS)
            nc.vector.memset(eps_t[:, :], EPS)
            nc.vector.memset(mln_t[:, :], float(-math.log(D_H)))
            nc.vector.memset(ones[:, :], 1.0)
            nc.vector.memset(ones_bf[:, :], 1.0)
            make_identity(nc, id128[:, :])
            nc.vector.tensor_copy(out=idbf[:, :], in_=id128[:, :])
            nc.sync.dma_start(out=kbc[:, :], in_=k_d[:].partition_broadcast(128))
            nc.sync.dma_start(out=gam[:, :], in_=gm_d[:].rearrange("(i p) -> p i", p=128))
            nc.sync.dma_start(out=bet[:, :], in_=bt_d[:].rearrange("(i p) -> p i", p=128))

            # ---------------- setup (transposes + Z precompute) -------------
            import os as _os
            _k_setup = int(_os.environ.get("K_SETUP", "3"))
            _k_head = int(_os.environ.get("K_HEAD", "1"))
            with ExitStack() as sctx:
                setup = sctx.enter_context(tc.tile_pool(name="setup", bufs=1))
                ps_st = sctx.enter_context(tc.tile_pool(name="ps_st", bufs=2, space="PSUM"))
                ps_z = sctx.enter_context(tc.tile_pool(name="ps_z", bufs=1, space="PSUM"))

                bh_sb = setup.tile([128, KI], fp32, tag="bh")
                nc.sync.dma_start(out=bh_sb[:, :], in_=bh_d[:].rearrange("(i p) -> p i", p=128))

                clean = setup.tile([BL, D_OUT], fp32, tag="clean")
                nc.sync.dma_start(out=clean[:, :], in_=cl_d[:, :])
                hbb = setup.tile([BL, D_OUT], fp32, tag="hbb")
                nc.sync.dma_start(out=hbb[:, :], in_=hb_d[:].partition_broadcast(BL))
                nc.vector.tensor_sub(hc[:, :], clean[:, :], hbb[:, :])
                scr_c = setup.tile([BL, D_OUT], fp32, tag="scrc")
                nc.vector.tensor_mul(scr_c[:, :], clean[:, :], clean[:, :])
                nc.vector.tensor_reduce(out=nrm[:, :], in_=scr_c[:, :],
                                        axis=AX.X, op=OP.add)
                nc.vector.tensor_scalar(out=nrm[:, :], in0=nrm[:, :],
                                        scalar1=1e-6, scalar2=None, op0=OP.add)
                nc.vector.reciprocal(out=nrm[:, :], in_=nrm[:, :])

                # z -> zT (d_g on partitions)
                z_n = setup.tile([T * BL, D_G], fp32, tag="zn")
                nc.sync.dma_start(out=z_n[:, :], in_=z_d[:, :])
                zT = setup.tile([128, 2, T * BL], fp32, tag="zT")
                if _k_setup < 1:
                    nc.vector.memset(zT[:, :, :], 0.0)
                else:
                    for ki in range(2):
                        tp = ps_st.tile([128, 128], fp32, tag="tp")
                        nc.tensor.transpose(tp[:, 0:T * BL],
                                            z_n[:, ki * 128:(ki + 1) * 128],
                                            id128[0:T * BL, 0:T * BL])
                        nc.scalar.copy(out=zT[:, ki, :], in_=tp[:, 0:T * BL])

                # W_g -> WgT
                wg_n = setup.tile([128, KI, D_G], fp32, tag="wgn")
                nc.sync.dma_start(out=wg_n[:, :, :],
                                  in_=wg_d[:, :].rearrange("(c p) g -> p c g", p=128))
                wgT = setup.tile([128, 2, D_H], fp32, tag="wgT")
                if _k_setup < 2:
                    nc.vector.memset(wgT[:, :, :], 0.0)
                    nc.vector.memset(hWT[:, :, :], 0.0)
                    nc.vector.memset(WhT[:, :, :], 0.0)
                else:
                    for co in range(KI):
                        for ki in range(2):
                            tp = ps_st.tile([128, 128], fp32, tag="tp")
                            nc.tensor.transpose(tp[:, :],
                                                wg_n[:, co, ki * 128:(ki + 1) * 128],
                                                id128[:, :])
                            nc.scalar.copy(out=wgT[:, ki, co * 128:(co + 1) * 128],
                                           in_=tp[:, :])

                    # head_W -> hWT
                    hw_n = setup.tile([128, 2, D_H], fp32, tag="hwn")
                    nc.sync.dma_start(out=hw_n[:, :, :],
                                      in_=hw_d[:, :].rearrange("(c p) d -> p c d", p=128))
                    for co in range(2):
                        for ki in range(KI):
                            tp = ps_st.tile([128, 128], fp32, tag="tp")
                            nc.tensor.transpose(tp[:, :],
                                                hw_n[:, co, ki * 128:(ki + 1) * 128],
                                                id128[:, :])
                            nc.scalar.copy(out=hWT[:, ki, co * 128:(co + 1) * 128],
                                           in_=tp[:, :])

                    # W_h -> WhT  (big one; on its own DMA queue)
                    wh_n = setup.tile([128, KI, D_H], fp32, tag="whn")
                    nc.gpsimd.dma_start(out=wh_n[:, :, :],
                                        in_=wh_d[:, :].rearrange("(c p) d -> p c d", p=128))
                    for co in range(KI):
                        for ki in range(KI):
                            tp = ps_st.tile([128, 128], fp32, tag="tp")
                            nc.tensor.transpose(tp[:, :],
                                                wh_n[:, co, ki * 128:(ki + 1) * 128],
                                                id128[:, :])
                            nc.scalar.copy(out=WhT[:, ki, co * 128:(co + 1) * 128],
                                           in_=tp[:, :])

                # Z[dout, (t,b)] = W_g z + b_h
                if _k_setup < 3:
                    nc.vector.memset(Z[:, :, :], 0.0)
                else:
                    for co in range(KI):
                        psz = ps_z.tile([128, T * BL], fp32, tag="psz")
                        for ki in range(2):
                            nc.tensor.matmul(psz[:, :],
                                             wgT[:, ki, co * 128:(co + 1) * 128],
                                             zT[:, ki, :],
                                             start=(ki == 0), stop=(ki == 1))
                        nc.vector.tensor_scalar(out=Z[:, co, :], in0=psz[:, :],
                                                scalar1=bh_sb[:, co:co + 1],
                                                scalar2=None, op0=OP.add)

            # Main-loop PSUM pools open after the setup pools close, so the
            # 8 banks are not shared across the two phases.
            ps_sm = ctx.enter_context(tc.tile_pool(name="ps_sm", bufs=4, space="PSUM"))
            ps_hb = ctx.enter_context(tc.tile_pool(name="ps_hb", bufs=1, space="PSUM"))
            ps_tp = ctx.enter_context(tc.tile_pool(name="ps_tp", bufs=2, space="PSUM"))
            ps_jk = ps_sm  # keep-warm (off by default) shares the small pool

            # ---------------- helpers ----------------
            _k_warm = int(_os.environ.get("K_WARM", "0"))

            def sview(m):           # S[:, m] as [p, b, i]
                return S[:, m, :, :]

            def warm(rhs_bf16=None, rhs_fp32=None):
                """Tiny matmul to keep the PE HAM un-throttled during
                DVE/ACT-heavy stretches. Reads a just-produced tile so Tile
                schedules it mid-step instead of sinking it."""
                if not _k_warm:
                    return
                jk = ps_jk.tile([8, 8], fp32, tag="jk")
                if rhs_bf16 is not None:
                    nc.tensor.matmul(jk[0:8, 0:2], ones_bf[:, 0:8], rhs_bf16,
                                     start=True, stop=True)
                else:
                    nc.tensor.matmul(jk[0:8, 0:2], ones[:, 0:8], rhs_fp32,
                                     start=True, stop=True)

            def ln_relu_block():
                """hs2[:,0] = h' -> layernorm+affine+relu -> S[:,0]."""
                nc.vector.tensor_mul(hs2[:, 1, :, :], hs2[:, 0, :, :], hs2[:, 0, :, :])
                r4 = work.tile([128, 2, BL], fp32, tag="r4")
                nc.vector.tensor_reduce(out=r4[:, :, :], in_=hs2[:, :, :, :],
                                        axis=AX.X, op=OP.add)
                st_ps = ps_sm.tile([128, 48], fp32, tag="sm")
                nc.tensor.matmul(st_ps[:, 0:4], ones[:, :],
                                 mkap(r4[:, :, :], [r4[:, :, :].ap[0], [1, 4]]),
                                 start=True, stop=True)
                warm(rhs_fp32=r4[:, 0, :])
                v = work.tile([128, 4], fp32, tag="v")
                nc.vector.tensor_scalar(out=v[:, :], in0=st_ps[:, 0:4],
                                        scalar1=1.0 / D_H, scalar2=None, op0=OP.mult)
                mq = work.tile([128, 2], fp32, tag="mq")
                nc.vector.tensor_mul(mq[:, :], v[:, 0:2], v[:, 0:2])
                nc.vector.tensor_sub(mq[:, :], v[:, 2:4], mq[:, :])
                # rstd = exp(-0.5 ln(var + eps))
                nc.scalar.activation(out=mq[:, :], in_=mq[:, :], func=AF.Ln,
                                     bias=lneps_t[:, :], scale=1.0)
                rs = work.tile([128, 2], fp32, tag="rs")
                nc.scalar.activation(out=rs[:, :], in_=mq[:, :], func=AF.Exp,
                                     bias=0.0, scale=-0.5)
                w1 = work.tile([128, BL, KI], fp32, tag="w1")
                nc.vector.tensor_sub(w1[:, :, :], hs2[:, 0, :, :],
                                     mkap(v[:, 0:2], [v[:, 0:2].ap[0], [1, 2], [0, KI]]))
                nc.vector.tensor_mul(w1[:, :, :], w1[:, :, :],
                                     mkap(rs[:, :], [rs[:, :].ap[0], [1, 2], [0, KI]]))
                nc.vector.tensor_mul(w1[:, :, :], w1[:, :, :],
                                     bcast_ax(gam[:, :], 1, BL))
                nc.vector.tensor_add(w1[:, :, :], w1[:, :, :],
                                     bcast_ax(bet[:, :], 1, BL))
                nc.vector.tensor_scalar(out=S[:, 0, :, :], in0=w1[:, :, :],
                                        scalar1=0.0, scalar2=None, op0=OP.max)

            def apply_A(rank):
                """S[:,1] = lam^(rank-1) * sum_s (w_s . hs) w_s  (uses hist[0:rank]).

                The lam^(rank-1) decay is baked into lam_ones (refreshed once
                per outer step) so pm is a plain tensor_tensor."""
                h_slice = hist[:, 0:rank, :, :]
                pm = work.tile([128, rank, BL, KI], fp32, tag="pm")
                nc.vector.tensor_mul(pm[:, :, :, :], h_slice,
                                     bcast_ax(sview(0), 1, rank))
                pr = work.tile([128, rank, BL], bf16, tag="pr")
                with nc.allow_low_precision("bf16 feed to fp32-accum ones-matmul"):
                    nc.vector.tensor_reduce(out=pr[:, :, :], in_=pm[:, :, :, :],
                                            axis=AX.X, op=OP.add)
                cp_ps = ps_sm.tile([128, 48], fp32, tag="sm")
                nc.tensor.matmul(cp_ps[:, 0:BL * rank], lam_ones[:, :],
                                 mkap(pr[:, :, :], [pr[:, :, :].ap[0], [1, BL * rank]]),
                                 start=True, stop=True)
                warm(rhs_bf16=pr[:, 0, :])
                qm = work.tile([128, BL, KI, rank], fp32, tag="qm")
                nc.vector.tensor_mul(qm[:, :, :, :],
                                     h_slice.rearrange("p r b i -> p b i r"),
                                     mkap(cp_ps[:, 0:BL * rank],
                                          [cp_ps[:, :].ap[0], [1, BL], [0, KI], [BL, rank]]))
                nc.vector.tensor_reduce(out=S[:, 1, :, :], in_=qm[:, :, :, :],
                                        axis=AX.X, op=OP.add)

            def inner_step(rank):
                apply_A(rank)
                # dots: D = [hs*Ah, Ah*Ah, hs*hs] -> reduce -> ones-MM
                D = work.tile([128, 3, BL, KI], fp32, tag="D")
                nc.vector.tensor_mul(D[:, 0:2, :, :], S[:, :, :, :],
                                     bcast_ax(sview(1), 1, 2))
                nc.vector.tensor_mul(D[:, 2, :, :], sview(0), sview(0))
                dd = work.tile([128, 3, BL], bf16, tag="dd")
                with nc.allow_low_precision("bf16 feed to fp32-accum ones-matmul"):
                    nc.vector.tensor_reduce(out=dd[:, :, :], in_=D[:, :, :, :],
                                            axis=AX.X, op=OP.add)
                dot_ps = ps_sm.tile([128, 48], fp32, tag="sm")
                nc.tensor.matmul(dot_ps[:, 0:6], ones_bf[:, :],
                                 mkap(dd[:, :, :], [dd[:, :, :].ap[0], [1, 6]]),
                                 start=True, stop=True)
                warm(rhs_bf16=dd[:, 0, :])
                # cols: d1=[0:2] (hs.Ah), d3=[2:4] (Ah.Ah), d2=[4:6] (hs.hs)
                # 1/(|hs||Ah|) = exp(-0.5 (ln d3 + ln d2)); ln clamped so a
                # zero Ah (d1=0 anyway) cannot produce inf*0.
                lns = work.tile([128, 4], fp32, tag="lns")
                nc.scalar.activation(out=lns[:, :], in_=dot_ps[:, 2:6],
                                     func=AF.Ln, bias=0.0, scale=1.0)
                lq = work.tile([128, 2], fp32, tag="lq")
                nc.vector.scalar_tensor_tensor(
                    out=lq[:, :], in0=lns[:, 0:2], scalar=-80.0,
                    in1=lns[:, 2:4], op0=OP.max, op1=OP.add)
                rdn = work.tile([128, 2], fp32, tag="rdn")
                nc.scalar.activation(out=rdn[:, :], in_=lq[:, :], func=AF.Exp,
                                     bias=0.0, scale=-0.5)
                rr = work.tile([128, 2], fp32, tag="rr")
                nc.vector.tensor_mul(rr[:, :], dot_ps[:, 0:2], rdn[:, :])
                nc.vector.tensor_scalar(out=rr[:, :], in0=rr[:, :],
                                        scalar1=0.0, scalar2=0.999999,
                                        op0=OP.max, op1=OP.min)
                nc.vector.tensor_scalar(out=rr[:, :], in0=rr[:, :],
                                        scalar1=-1.0, scalar2=1.0,
                                        op0=OP.mult, op1=OP.add)     # om = 1-R
                lg = work.tile([128, 2], fp32, tag="lg")
                nc.scalar.activation(out=lg[:, :], in_=rr[:, :], func=AF.Ln,
                                     bias=0.0, scale=1.0)
                ek = work.tile([128, 2], fp32, tag="ek")
                nc.scalar.activation(out=ek[:, :], in_=lg[:, :], func=AF.Exp,
                                     bias=0.0, scale=kbc[:, 0:1])    # (1-R)^k
                # as2 = [1-ek, 2-ek] = [a | .], then as2[:,2:4] *= ek -> 1-a^2
                as2 = work.tile([128, 4], fp32, tag="as2")
                nc.vector.tensor_sub(as2[:, :], cs12[:, :],
                                     mkap(ek[:, :], [ek[:, :].ap[0], [0, 2], [1, 2]]))
                nc.vector.tensor_mul(as2[:, 2:4], as2[:, 2:4], ek[:, :])
                u1 = work.tile([128, BL, KI], fp32, tag="u1")
                nc.vector.tensor_mul(u1[:, :, :], sview(1),
                                     mkap(as2[:, 0:2], [as2[:, :].ap[0], [1, 2], [0, KI]]))
                u2 = work.tile([128, BL, KI], fp32, tag="u2")
                nc.vector.tensor_mul(u2[:, :, :], hbase[:, :, :],
                                     mkap(as2[:, 2:4],
                                          [as2[:, 2:4].ap[0], [1, 2], [0, KI]]))
                nc.vector.tensor_add(hs2[:, 0, :, :], u1[:, :, :], u2[:, :, :])
                ln_relu_block()

            def append(t):
                """hist[t] = S[:,0] * sqrt(ETA * LAM^-t / (|h|^2+EPS))."""
                u1 = work.tile([128, BL, KI], fp32, tag="u1")
                nc.vector.tensor_mul(u1[:, :, :], sview(0), sview(0))
                r2 = work.tile([128, BL], bf16, tag="r2")
                with nc.allow_low_precision("bf16 feed to fp32-accum ones-matmul"):
                    nc.vector.tensor_reduce(out=r2[:, :], in_=u1[:, :, :],
                                            axis=AX.X, op=OP.add)
                hn_ps = ps_sm.tile([128, 48], fp32, tag="sm")
                nc.tensor.matmul(hn_ps[:, 0:BL], ones_bf[:, :], r2[:, :],
                                 start=True, stop=True)
                sd = work.tile([128, 2], fp32, tag="sd")
                nc.scalar.activation(out=sd[:, :], in_=hn_ps[:, 0:BL],
                                     func=AF.Ln, bias=eps_t[:, :], scale=1.0)
                iv = work.tile([128, 2], fp32, tag="iv")
                nc.scalar.activation(out=iv[:, :], in_=sd[:, :], func=AF.Exp,
                                     bias=0.0, scale=-0.5)
                nc.vector.scalar_tensor_tensor(
                    out=hist[:, t, :, :], in0=sview(0),
                    scalar=float(math.sqrt(ETA * LAM ** (-t))),
                    in1=mkap(iv[:, :], [iv[:, :].ap[0], [1, 2], [0, KI]]),
                    op0=OP.mult, op1=OP.mult)

            def h_base_step(t):
                """hbase = W_h @ S[:,0] + Z[:, :, t] ; S[:,0] = relu(hbase)."""
                with nc.allow_low_precision("bf16 matmul inputs, fp32 accum"):
                    nc.vector.tensor_copy(out=hs_bf[:, :, :], in_=S[:, 0, :, :])
                for half in range(2):
                    hb_ps = ps_hb.tile([BL, 384], fp32, tag=f"hb{half}")
                    for ki in range(KI):
                        nc.tensor.matmul(hb_ps[:, :], hs_bf[:, :, ki],
                                         WhT[:, ki, half * 384:(half + 1) * 384],
                                         start=(ki == 0), stop=(ki == KI - 1))
                    if half == 0:
                        with nc.allow_low_precision("bf16 transpose staging"):
                            nc.scalar.copy(out=hbT[:, 0:384], in_=hb_ps[:, :])
                    else:
                        with nc.allow_low_precision("bf16 transpose staging"):
                            nc.vector.tensor_copy(out=hbT[:, 384:768], in_=hb_ps[:, :])
                htp = ps_tp.tile([128, KI, BL], bf16, tag="htp")
                for ki in range(KI):
                    nc.tensor.transpose(htp[:, ki, :],
                                        hbT[:, ki * 128:(ki + 1) * 128],
                                        idbf[0:BL, 0:BL])
                zt = Z[:, :, BL * t:BL * (t + 1)].rearrange("p i b -> p b i")
                nc.vector.tensor_add(hbase[:, :, :],
                                     htp[:, :, :].rearrange("p i b -> p b i"), zt)
                nc.vector.tensor_scalar(out=S[:, 0, :, :], in0=hbase[:, :, :],
                                        scalar1=0.0, scalar2=None, op0=OP.max)

            # ---------------- time loop ----------------
            import os as _os
            _n_t0 = int(_os.environ.get("K_T0", "1"))
            _n_mid = int(_os.environ.get("K_MID", str(T - 2)))
            _n_fin = int(_os.environ.get("K_FIN", "1"))

            # t = 0: A=0 -> h = relu(LN(Z_0))
            if _n_t0:
                nc.vector.tensor_copy(
                    out=hs2[:, 0, :, :],
                    in_=Z[:, :, 0:BL].rearrange("p i b -> p b i"))
                ln_relu_block()
                append(0)
            else:
                nc.vector.memset(S[:, :, :, :], 0.0)
                nc.vector.memset(hbase[:, :, :], 0.0)

            for t in range(1, 1 + _n_mid):
                h_base_step(t)
                nc.vector.memset(lam_ones[:, :], float(LAM ** (t - 1)))
                for _ in range(S_IN):
                    inner_step(t)
                append(t)

            # final step: h_s = relu(LN(h_base + A h_s))
            if _n_fin:
                h_base_step(T - 1)
                nc.vector.memset(lam_ones[:, :], float(LAM ** (RANK - 1)))
                for _ in range(S_IN):
                    apply_A(RANK)
                    nc.vector.tensor_add(hs2[:, 0, :, :], hbase[:, :, :], sview(1))
                    ln_relu_block()

            # head + loss
            lb = work.tile([BL, 1], fp32, tag="lb")
            if _k_head:
                pred_ps = ps_hb.tile([BL, D_OUT], fp32, tag="hb0")
                for ki in range(KI):
                    nc.tensor.matmul(pred_ps[:, :], S[:, 0, :, ki], hWT[:, ki, :],
                                     start=(ki == 0), stop=(ki == KI - 1))
                df = work.tile([BL, D_OUT], fp32, tag="df")
                nc.vector.tensor_sub(df[:, :], pred_ps[:, :], hc[:, :])
                df2 = work.tile([BL, D_OUT], fp32, tag="df2")
                se = work.tile([BL, 1], fp32, tag="se")
                nc.vector.tensor_mul(df2[:, :], df[:, :], df[:, :])
                nc.vector.tensor_reduce(out=se[:, :], in_=df2[:, :],
                                        axis=AX.X, op=OP.add)
                nc.vector.tensor_mul(se[:, :], se[:, :], nrm[:, :])
                nc.scalar.activation(out=lb[:, :], in_=se[:, :], func=AF.Ln,
                                     bias=1.0, scale=1.0)
            else:
                nc.vector.memset(lb[:, :], 0.0)
            nc.sync.dma_start(out=loss_d[:, :], in_=lb[:, :])

    nc.compile()
    return nc


def _get_nc():
    if "nc" not in _CACHE:
        _CACHE["nc"] = _build()
    return _CACHE["nc"]


def _numpy_kernel(z_seq, clean_vec, W_h, W_g, b_h, alpha_fw, ln_gamma, ln_beta,
                  head_W, head_b):
    def _layernorm(x, g, b):
        mu = np.mean(x, axis=-1, keepdims=True)
        var = np.mean((x - mu) ** 2, axis=-1, keepdims=True)
        return g * (x - mu) / np.sqrt(var + LN_EPS) + b

    k = _compute_k(np.asarray(alpha_fw).reshape(()))
    h = np.zeros((B, D_H), np.float32)
    hist = np.zeros((B, T - 1, D_H), np.float32)
    coef = np.zeros((B, T - 1), np.float32)
    rank = 0
    W_hT = W_h.T.copy()
    Z = (z_seq.reshape(T * B, D_G) @ W_g.T).reshape(T, B, D_H) + b_h

    def apply_A(x):
        if rank == 0:
            return np.zeros_like(x)
        Hr = hist[:, :rank, :]
        proj = np.matmul(Hr, x[:, :, None])[:, :, 0]
        return np.matmul((coef[:, :rank] * proj)[:, None, :], Hr)[:, 0, :]

    for t in range(T - 1):
        h_base = h @ W_hT + Z[t]
        h_s = np.maximum(h_base, 0.0)
        for _ in range(S_IN):
            Ah = apply_A(h_s)
            dot = np.sum(h_s * Ah, axis=1, keepdims=True)
            n1 = np.linalg.norm(h_s, axis=1, keepdims=True) + 1e-6
            n2 = np.linalg.norm(Ah, axis=1, keepdims=True) + 1e-6
            R_pos = np.clip(dot / (n1 * n2 + 1e-6), 0.0, 1.0)
            a = 1.0 - (1.0 - R_pos) ** k
            h_s = (1.0 - a ** 2) * h_base + a * Ah
            h_s = np.maximum(_layernorm(h_s, ln_gamma, ln_beta), 0.0)
        h = h_s
        hn2 = np.sum(h * h, axis=1) + EPS
        coef[:, :rank] *= LAM
        coef[:, rank] = ETA / hn2
        hist[:, rank, :] = h
        rank += 1

    h_base = h @ W_hT + Z[T - 1]
    h_s = np.maximum(h_base, 0.0)
    for _ in range(S_IN):
        h_s = np.maximum(_layernorm(h_base + apply_A(h_s), ln_gamma, ln_beta), 0.0)

    pred = h_s @ head_W.T + head_b
    diff = pred - clean_vec
    per_sample_se = np.sum(diff ** 2, axis=1)
    norm_clean = np.sum(clean_vec ** 2, axis=1) + 1e-6
    rel_err = per_sample_se / norm_clean
    return np.asarray(np.mean(np.log1p(rel_err)), np.float32)


def _make_in_maps(z_seq, clean_vec, W_h, W_g, b_h, k, ln_gamma, ln_beta,
                  head_W, head_b):
    in_maps = []
    for c in range(NCORES):
        sl = slice(c * BL, (c + 1) * BL)
        in_maps.append({
            "z": np.ascontiguousarray(
                z_seq[:, sl, :].reshape(T * BL, D_G), np.float32),
            "clean": np.ascontiguousarray(clean_vec[sl], np.float32),
            "W_h": np.asarray(W_h, np.float32),
            "W_g": np.asarray(W_g, np.float32),
            "b_h": np.asarray(b_h, np.float32),
            "ln_gamma": np.asarray(ln_gamma, np.float32),
            "ln_beta": np.asarray(ln_beta, np.float32),
            "head_W": np.asarray(head_W, np.float32),
            "head_b": np.asarray(head_b, np.float32),
            "k_fw": np.asarray([k], np.float32),
        })
    return in_maps


def run_on_hw(inputs, trace=False, **kw):
    """Build + run on the 8 NeuronCores. Returns (loss, BassKernelResults)."""
    from concourse.bass_utils import run_bass_kernel_spmd
    nc = _get_nc()
    k = _compute_k(np.asarray(inputs["alpha_fw"]).reshape(()))
    in_maps = _make_in_maps(
        inputs["z_seq"], inputs["clean_vec"], inputs["W_h"], inputs["W_g"],
        inputs["b_h"], k, inputs["ln_gamma"], inputs["ln_beta"],
        inputs["head_W"], inputs["head_b"])
    res = run_bass_kernel_spmd(nc, in_maps, list(range(NCORES)),
                               trace=trace, **kw)
    losses = np.concatenate(
        [np.asarray(r["loss"]).reshape(-1) for r in res.results])
    return np.asarray(np.mean(losses), np.float32), res


def kernel(z_seq, clean_vec, W_h, W_g, b_h, alpha_fw, ln_gamma, ln_beta,
           head_W, head_b):
    inputs = dict(z_seq=z_seq, clean_vec=clean_vec, W_h=W_h, W_g=W_g, b_h=b_h,
                  alpha_fw=alpha_fw, ln_gamma=ln_gamma, ln_beta=ln_beta,
                  head_W=head_W, head_b=head_b)
    try:
        loss, _ = run_on_hw(inputs)
        return loss
    except Exception:
        return _numpy_kernel(
            np.asarray(z_seq, np.float32), np.asarray(clean_vec, np.float32),
            np.asarray(W_h, np.float32), np.asarray(W_g, np.float32),
            np.asarray(b_h, np.float32), alpha_fw,
            np.asarray(ln_gamma, np.float32), np.asarray(ln_beta, np.float32),
            np.asarray(head_W, np.float32), np.asarray(head_b, np.float32))



# revision 18
# speedup vs baseline: 1.2615x; 1.2615x over previous
"""Self-contained Trainium2 (Bass/Tile) kernel for nn_CoreRNNFW_65463891525848.

Strategy
--------
Pure data parallel: batch B=16 is sharded 2-per-core across 8 NeuronCores;
weights are replicated.  Each core runs the full T*S recurrence for its two
samples inside ONE fully-unrolled Bass program.

Layout ("64p"): every length-768 vector lives as [64 partitions, 12 free],
with sample b occupying partitions [64b, 64b+64).  d = q + 64*j.  This lets
per-sample scalars live as [128,1] per-partition columns (usable as
tensor_scalar / scalar_tensor_tensor scalar operands and activation
bias/scale), per-sample cross-partition sums go through ONE block-diagonal
ones matmul, and the whole LayerNorm collapses into a single fused GPSIMD
instruction (n_tokens=2, dmodel = 64*12 = 768).

Key algebraic moves vs. a naive lowering:
- A is kept in factored form  A_t = lam^t * W W^T  with  w_s = v_s *
  sqrt(eta lam^-s / n_s); lam^(t-1) is folded into the `a` coefficient
  (R is scale-invariant in A up to 1e-6 epsilons).
- dot = h.Ah = |W^T h|^2 = |proj|^2  -> one Square-activation with
  accumulator, no 768-dim dot needed.
- a(R) = 1-(1-R)^k and g(R) = 1-a(R)^2 are evaluated as degree-10
  polynomials via ONE tensor_tensor_scan (Horner) each; coefficients are
  fitted host-side from the runtime alpha_fw and shipped as inputs.
- |h|^2 partials for the *next* inner step are computed right after the
  relu, off the critical path; one block-ones matmul then yields
  [AA | hh] together.

A numpy fallback keeps kernel() correct for inputs the fast path does not
cover (e.g. alpha values whose a(R) is not polynomial-fittable).
"""

import math
import numpy as np

T, B = 24, 16
NCORES = 8
BL = B // NCORES          # 2 samples per core
D_G, D_H, D_OUT = 256, 768, 256
KI = D_H // 128           # 6 chunks of the hidden dim (128p e-layout)
FJ = D_H // 64            # 12 free slots of the hidden dim (64p layout)
S_IN = 4
LAM = 0.95
ETA = 0.5
EPS = 1e-6
LN_EPS = 1e-5
RANK = T - 1              # 23 hebbian updates
NDEG = 10                 # polynomial degree for a(R), g(R)
ND = NDEG + 1


def _softplus(x):
    return np.log1p(np.exp(-abs(float(x)))) + max(float(x), 0.0)


def _compute_k(alpha):
    a = float(alpha)
    if a >= 0:
        return np.float32(1.0 + _softplus(a))
    return np.float32(1.0 / (1.0 + _softplus(-a)))


def _fit_coefs(k):
    """Descending power-basis coefs of deg-NDEG fits to a(R), g(R) on [0,1].

    Returns (acoef, gcoef, max_fit_err). Scan evaluates
    state=0; for t: state = R*state + coef[t] -> p(R) with coef[0] leading.
    """
    from numpy.polynomial import chebyshev as C
    from numpy.polynomial import Polynomial

    def f_a(R):
        return 1.0 - (1.0 - R) ** k

    def f_g(R):
        a = f_a(R)
        return 1.0 - a * a

    ca = C.Chebyshev.interpolate(f_a, NDEG, domain=[0.0, 1.0]).convert(
        kind=Polynomial).coef
    cg = C.Chebyshev.interpolate(f_g, NDEG, domain=[0.0, 1.0]).convert(
        kind=Polynomial).coef
    R = np.linspace(0.0, 1.0, 4001)

    def horner32(c, x):
        s = np.zeros_like(x, dtype=np.float32)
        for cc in c[::-1]:
            s = s * x.astype(np.float32) + np.float32(cc)
        return s

    err = max(np.abs(horner32(ca, R) - f_a(R)).max(),
              np.abs(horner32(cg, R) - f_g(R)).max())
    acoef = np.ascontiguousarray(ca[::-1], np.float32)   # descending
    gcoef = np.ascontiguousarray(cg[::-1], np.float32)
    return acoef, gcoef, float(err)


# ----------------------------------------------------------------------------
# Bass program
# ----------------------------------------------------------------------------

_CACHE = {}


def _build():
    import concourse.bass as bass
    import concourse.tile as tile
    from concourse import bacc, mybir, library_config
    from concourse.masks import make_identity

    # Pin all ACT functions (Ln/Exp/Copy/Square) to one table set so bacc
    # never emits mid-kernel ACT_TABLE_LOADs (~1.3us each).
    if not getattr(bacc, "_act_tables_pinned", False):
        _orig_get_tables = bacc.get_activation_tables

        def _pinned_tables(arch):
            tabs = _orig_get_tables(arch)
            anchor = tabs.get("natural_log_exp_and_others")
            if anchor:
                for name in tabs:
                    if name != "natural_log_exp_and_others":
                        tabs[name] = tabs[name] - anchor
            return tabs

        bacc.get_activation_tables = _pinned_tables
        bacc._act_tables_pinned = True

    fp32 = mybir.dt.float32
    bf16 = mybir.dt.bfloat16
    AX = mybir.AxisListType
    OP = mybir.AluOpType
    AF = mybir.ActivationFunctionType

    def mkap(base, pairs):
        return bass.AP(tensor=base.tensor, offset=base.offset,
                       ap=[list(p) for p in pairs])

    def bcast_ax(ap_obj, pos, count):
        """Insert a 0-stride (broadcast) axis at `pos` (0 = partition dim)."""
        pairs = [list(p) for p in ap_obj.ap]
        pairs.insert(pos, [0, count])
        return bass.AP(tensor=ap_obj.tensor, offset=ap_obj.offset, ap=pairs)

    nc = bacc.Bacc("TRN2", target_bir_lowering=False, debug=False)

    z_d = nc.declare_dram_parameter("z", [T * BL, D_G], fp32, isOutput=False)
    cl_d = nc.declare_dram_parameter("clean", [BL, D_OUT], fp32, isOutput=False)
    wh_d = nc.declare_dram_parameter("W_h", [D_H, D_H], fp32, isOutput=False)
    wg_d = nc.declare_dram_parameter("W_g", [D_H, D_G], fp32, isOutput=False)
    bh_d = nc.declare_dram_parameter("b_h", [D_H], fp32, isOutput=False)
    gm_d = nc.declare_dram_parameter("ln_gamma", [D_H], fp32, isOutput=False)
    bt_d = nc.declare_dram_parameter("ln_beta", [D_H], fp32, isOutput=False)
    hw_d = nc.declare_dram_parameter("head_W", [D_OUT, D_H], fp32, isOutput=False)
    hb_d = nc.declare_dram_parameter("head_b", [D_OUT], fp32, isOutput=False)
    ac_d = nc.declare_dram_parameter("acoef", [ND], fp32, isOutput=False)
    gc_d = nc.declare_dram_parameter("gcoef", [ND], fp32, isOutput=False)
    loss_d = nc.declare_dram_parameter("loss", [BL, 1], fp32, isOutput=True)

    with tile.TileContext(nc) as tc:
        from contextlib import ExitStack
        with ExitStack() as ctx:
            persist = ctx.enter_context(tc.tile_pool(name="persist", bufs=1))
            work = ctx.enter_context(tc.tile_pool(name="work", bufs=3))

            # ---------------- persistent state ----------------
            bones = persist.tile([128, 128], bf16, tag="bones")
            id128 = persist.tile([128, 128], fp32, tag="id128")
            idbf = persist.tile([128, 128], bf16, tag="idbf")
            WhT = persist.tile([128, KI, D_H], bf16, tag="WhT")
            hWT = persist.tile([128, KI, D_OUT], fp32, tag="hWT")
            gam64 = persist.tile([128, FJ], fp32, tag="gam64")
            bet64 = persist.tile([128, FJ], fp32, tag="bet64")
            acoef = persist.tile([128, ND], fp32, tag="acoef")
            gcoef = persist.tile([128, ND], fp32, tag="gcoef")
            hist = persist.tile([128, RANK, FJ], bf16, tag="hist")
            hs64 = persist.tile([128, FJ], bf16, tag="hs64")
            hs128 = persist.tile([128, BL, KI], bf16, tag="hs128")
            hb64 = persist.tile([128, FJ], fp32, tag="hb64")
            ln64 = persist.tile([128, FJ], fp32, tag="ln64")
            h1 = persist.tile([128, FJ], fp32, tag="h1")
            aahh = persist.tile([128, 2], bf16, tag="aahh")   # [AAp | hhp]
            dotv = persist.tile([128, 1], fp32, tag="dotv")
            hbT = persist.tile([BL, D_H], bf16, tag="hbT")
            hc = persist.tile([BL, D_OUT], fp32, tag="hc")    # clean - head_b
            nrm = persist.tile([BL, 1], fp32, tag="nrm")
            eps_t = persist.tile([128, 1], fp32, tag="epst")
            nc.vector.memset(eps_t[:, :], EPS)
            Z64 = persist.tile([128, T, FJ], fp32, tag="Z64")

            # block-diag ones: two 64x64 all-ones blocks
            nc.vector.memset(bones[:, :], 0.0)
            nc.vector.memset(bones[0:64, 0:64], 1.0)
            nc.vector.memset(bones[64:128, 64:128], 1.0)
            make_identity(nc, id128[:, :])
            nc.vector.tensor_copy(out=idbf[:, :], in_=id128[:, :])
            # swap-halves permutation: swap64[p, o] = 1 iff o == (p+64)%128.
            # PE-multiplying by it moves partition p's row to partition p+-64.
            swap64 = persist.tile([128, 128], bf16, tag="swap64")
            sw32 = persist.tile([128, 128], fp32, tag="sw32")
            nc.gpsimd.memset(sw32[:, :], 0.0)
            for base_off in (-64, 64):
                nc.gpsimd.affine_select(
                    out=sw32[:, :], in_=sw32[:, :],
                    compare_op=mybir.AluOpType.not_equal,
                    fill=1.0, base=base_off,
                    pattern=[[-1, 128]], channel_multiplier=1)
            nc.vector.tensor_copy(out=swap64[:, :], in_=sw32[:, :])
            nc.sync.dma_start(out=acoef[:, :],
                              in_=ac_d[:].partition_broadcast(128))
            nc.sync.dma_start(out=gcoef[:, :],
                              in_=gc_d[:].partition_broadcast(128))
            # gamma/beta in 64p layout: d = q + 64j -> [q, j]; same for both
            # samples (partition halves).
            for pb in range(2):
                nc.sync.dma_start(
                    out=gam64[64 * pb:64 * (pb + 1), :],
                    in_=gm_d[:].rearrange("(j q) -> q j", q=64))
                nc.sync.dma_start(
                    out=bet64[64 * pb:64 * (pb + 1), :],
                    in_=bt_d[:].rearrange("(j q) -> q j", q=64))

            # GPSIMD: identity/affine_select above run on the default
            # library; switch to `attn` (has the fused layernorm) for the
            # rest of the kernel.
            nc.gpsimd.load_library(library_config.attn)

            # ---------------- setup (transposes + Z precompute) -------------
            with ExitStack() as sctx:
                setup = sctx.enter_context(tc.tile_pool(name="setup", bufs=1))
                ps_st = sctx.enter_context(
                    tc.tile_pool(name="ps_st", bufs=2, space="PSUM"))
                ps_z = sctx.enter_context(
                    tc.tile_pool(name="ps_z", bufs=1, space="PSUM"))

                bh_sb = setup.tile([128, KI], fp32, tag="bh")
                nc.sync.dma_start(out=bh_sb[:, :],
                                  in_=bh_d[:].rearrange("(i p) -> p i", p=128))

                clean = setup.tile([BL, D_OUT], fp32, tag="clean")
                nc.sync.dma_start(out=clean[:, :], in_=cl_d[:, :])
                hbb = setup.tile([BL, D_OUT], fp32, tag="hbb")
                nc.sync.dma_start(out=hbb[:, :],
                                  in_=hb_d[:].partition_broadcast(BL))
                nc.vector.tensor_sub(hc[:, :], clean[:, :], hbb[:, :])
                scr_c = setup.tile([BL, D_OUT], fp32, tag="scrc")
                nc.vector.tensor_mul(scr_c[:, :], clean[:, :], clean[:, :])
                nc.vector.tensor_reduce(out=nrm[:, :], in_=scr_c[:, :],
                                        axis=AX.X, op=OP.add)
                nc.vector.tensor_scalar(out=nrm[:, :], in0=nrm[:, :],
                                        scalar1=1e-6, scalar2=None, op0=OP.add)
                nc.vector.reciprocal(out=nrm[:, :], in_=nrm[:, :])

                # z -> zT (d_g on partitions)
                z_n = setup.tile([T * BL, D_G], fp32, tag="zn")
                nc.sync.dma_start(out=z_n[:, :], in_=z_d[:, :])
                zT = setup.tile([128, 2, T * BL], fp32, tag="zT")
                for ki in range(2):
                    tp = ps_st.tile([128, 128], fp32, tag="tp")
                    nc.tensor.transpose(tp[:, 0:T * BL],
                                        z_n[:, ki * 128:(ki + 1) * 128],
                                        id128[0:T * BL, 0:T * BL])
                    nc.scalar.copy(out=zT[:, ki, :], in_=tp[:, 0:T * BL])

                # W_g -> WgT
                wg_n = setup.tile([128, KI, D_G], fp32, tag="wgn")
                nc.sync.dma_start(out=wg_n[:, :, :],
                                  in_=wg_d[:, :].rearrange("(c p) g -> p c g", p=128))
                wgT = setup.tile([128, 2, D_H], fp32, tag="wgT")
                for co in range(KI):
                    for ki in range(2):
                        tp = ps_st.tile([128, 128], fp32, tag="tp")
                        nc.tensor.transpose(tp[:, :],
                                            wg_n[:, co, ki * 128:(ki + 1) * 128],
                                            id128[:, :])
                        nc.scalar.copy(out=wgT[:, ki, co * 128:(co + 1) * 128],
                                       in_=tp[:, :])

                # head_W -> hWT
                hw_n = setup.tile([128, 2, D_H], fp32, tag="hwn")
                nc.sync.dma_start(out=hw_n[:, :, :],
                                  in_=hw_d[:, :].rearrange("(c p) d -> p c d", p=128))
                for co in range(2):
                    for ki in range(KI):
                        tp = ps_st.tile([128, 128], fp32, tag="tp")
                        nc.tensor.transpose(tp[:, :],
                                            hw_n[:, co, ki * 128:(ki + 1) * 128],
                                            id128[:, :])
                        nc.scalar.copy(out=hWT[:, ki, co * 128:(co + 1) * 128],
                                       in_=tp[:, :])

                # W_h -> WhT
                wh_n = setup.tile([128, KI, D_H], fp32, tag="whn")
                nc.sync.dma_start(out=wh_n[:, :, :],
                                  in_=wh_d[:, :].rearrange("(c p) d -> p c d", p=128))
                for co in range(KI):
                    for ki in range(KI):
                        tp = ps_st.tile([128, 128], fp32, tag="tp")
                        nc.tensor.transpose(tp[:, :],
                                            wh_n[:, co, ki * 128:(ki + 1) * 128],
                                            id128[:, :])
                        nc.scalar.copy(out=WhT[:, ki, co * 128:(co + 1) * 128],
                                       in_=tp[:, :])

                # Z[dout, (t,b)] = W_g z + b_h   (128p e-layout)
                Z = setup.tile([128, KI, T * BL], fp32, tag="Z")
                for co in range(KI):
                    psz = ps_z.tile([128, T * BL], fp32, tag="psz")
                    for ki in range(2):
                        nc.tensor.matmul(psz[:, :],
                                         wgT[:, ki, co * 128:(co + 1) * 128],
                                         zT[:, ki, :],
                                         start=(ki == 0), stop=(ki == 1))
                    nc.vector.tensor_scalar(out=Z[:, co, :], in0=psz[:, :],
                                            scalar1=bh_sb[:, co:co + 1],
                                            scalar2=None, op0=OP.add)

                # Convert Z (e-layout) -> Z64 (64p layout) with 4 sbuf->sbuf
                # DMAs (DMA may cross partitions; compute engines may not).
                # Z64[64b+q, t, j] = Z[q + 64*(j%2), j//2, t*BL+b]
                for b in range(BL):
                    for ph in range(2):
                        for c in range(KI):
                            zin = Z[64 * ph:64 * (ph + 1), c, b]
                            zin2 = mkap(zin, [zin.ap[0], [BL, T]])
                            zo2 = Z64[64 * b:64 * (b + 1), :, ph + 2 * c]
                            nc.sync.dma_start(out=zo2, in_=zin2)

            # Main-loop PSUM pools (setup pools closed above).
            ps_cp = ctx.enter_context(tc.tile_pool(name="ps_cp", bufs=1, space="PSUM"))
            ps_dn = ctx.enter_context(tc.tile_pool(name="ps_dn", bufs=1, space="PSUM"))
            ps_hb = ctx.enter_context(tc.tile_pool(name="ps_hb", bufs=1, space="PSUM"))
            ps_tp = ctx.enter_context(tc.tile_pool(name="ps_tp", bufs=1, space="PSUM"))

            # ---------------- helpers ----------------

            def j2(src_tile, p0, jo):
                """[64, KI] view: partitions [p0, p0+64), free j = jo, jo+2..."""
                base = src_tile[p0:p0 + 64, jo:FJ]
                return mkap(base, [base.ap[0], [base.ap[1][0] * 2, KI]])

            def relu_to_hs64(src):
                nc.vector.tensor_scalar(out=hs64[:, :], in0=src,
                                        scalar1=0.0, scalar2=None, op0=OP.max)

            def make_hs128():
                """hs64 (64p) -> hs128 (128p e-layout) via one swap-matmul.

                hs128[p, b, c] = hs64[64b + p%64, p//64 + 2c]; the two
                cross-half blocks read the PE-swapped copy."""
                swp2 = ps_tp.tile([128, FJ], fp32, tag="swp")
                nc.tensor.matmul(swp2[:, :], swap64[:, :], hs64[:, :],
                                 start=True, stop=True)
                nc.vector.tensor_copy(out=hs128[0:64, 0, :], in_=j2(hs64, 0, 0))
                nc.vector.tensor_copy(out=hs128[64:128, 0, :], in_=j2(swp2, 64, 1))
                nc.vector.tensor_copy(out=hs128[0:64, 1, :], in_=j2(swp2, 0, 0))
                nc.vector.tensor_copy(out=hs128[64:128, 1, :], in_=j2(hs64, 64, 1))

            def hh_prep():
                """aahh[:,1] = per-partition partial of |h_s|^2 (bf16)."""
                scr = work.tile([128, FJ], bf16, tag="hhscr")
                with nc.allow_low_precision("bf16 partials into block-ones matmul"):
                    nc.vector.scalar_tensor_tensor(
                        out=scr[:, :], in0=hs64[:, :], scalar=1.0,
                        in1=hs64[:, :], op0=OP.mult, op1=OP.mult,
                        accum_out=aahh[:, 1:2])

            def do_layernorm():
                nc.gpsimd.layernorm(ln64[:, :], h1[:, :],
                                    gamma_ap=gam64[:, :], beta_ap=bet64[:, :],
                                    eps=LN_EPS, subtract_mean=True, n_tokens=2)

            def apply_A_to_Ah(rank):
                """Ah (fp32 [128, FJ], unscaled by lam) + dotv = |proj|^2.

                Returns the Ah tile."""
                h_slice = hist[:, 0:rank, :]
                pm = work.tile([128, rank, FJ], bf16, tag="pm")
                nc.vector.tensor_mul(pm[:, :, :], h_slice,
                                     bcast_ax(hs64[:, :], 1, rank))
                pr = work.tile([128, rank], bf16, tag="pr")
                with nc.allow_low_precision("bf16 feed to block-ones matmul"):
                    nc.vector.tensor_reduce(out=pr[:, :], in_=pm[:, :, :],
                                            axis=AX.X, op=OP.add)
                cp = ps_cp.tile([128, RANK], fp32, tag="cp")
                nc.tensor.matmul(cp[:, 0:rank], bones[:, :], pr[:, :],
                                 start=True, stop=True)
                # dot = |proj|^2 via Square-activation accumulator (ACT engine)
                sqs = work.tile([128, RANK], fp32, tag="sqs")
                nc.scalar.activation(out=sqs[:, 0:rank], in_=cp[:, 0:rank],
                                     func=AF.Square, accum_out=dotv[:, :])
                qm = work.tile([128, FJ, rank], bf16, tag="qm")
                nc.vector.tensor_mul(
                    qm[:, :, :],
                    h_slice.rearrange("p r f -> p f r"),
                    mkap(cp[:, 0:rank],
                         [cp[:, 0:rank].ap[0], [0, FJ], [1, rank]]))
                Ah = work.tile([128, FJ], fp32, tag="Ah")
                nc.vector.tensor_reduce(out=Ah[:, :], in_=qm[:, :, :],
                                        axis=AX.X, op=OP.add)
                return Ah

            def inner_step(rank, lam_pow):
                Ah = apply_A_to_Ah(rank)
                # AA partials; one block-ones matmul gives [AA | hh] full sums
                aasc = work.tile([128, FJ], bf16, tag="aascr")
                with nc.allow_low_precision("bf16 partials into block-ones matmul"):
                    nc.vector.scalar_tensor_tensor(
                        out=aasc[:, :], in0=Ah[:, :], scalar=1.0,
                        in1=Ah[:, :], op0=OP.mult, op1=OP.mult,
                        accum_out=aahh[:, 0:1])
                dn = ps_dn.tile([128, 2], fp32, tag="dn")
                nc.tensor.matmul(dn[:, :], bones[:, :], aahh[:, :],
                                 start=True, stop=True)
                # rden = (AA*hh)^-1/2 = exp(-0.5*(ln AA + ln hh))
                lns = work.tile([128, 2], fp32, tag="lns")
                nc.scalar.activation(out=lns[:, :], in_=dn[:, :], func=AF.Ln,
                                     bias=0.0, scale=1.0)
                lq = work.tile([128, 1], fp32, tag="lq")
                nc.vector.scalar_tensor_tensor(
                    out=lq[:, :], in0=lns[:, 0:1], scalar=-80.0,
                    in1=lns[:, 1:2], op0=OP.max, op1=OP.add)
                rden = work.tile([128, 1], fp32, tag="rden")
                nc.scalar.activation(out=rden[:, :], in_=lq[:, :], func=AF.Exp,
                                     bias=0.0, scale=-0.5)
                rr = work.tile([128, 1], fp32, tag="rr")
                nc.vector.tensor_mul(rr[:, :], dotv[:, :], rden[:, :])
                Rcl = work.tile([128, 1], fp32, tag="Rcl")
                nc.vector.tensor_scalar(out=Rcl[:, :], in0=rr[:, :],
                                        scalar1=0.0, scalar2=1.0,
                                        op0=OP.max, op1=OP.min)
                # a(R), g(R)=1-a^2 via Horner scans
                Rb = mkap(Rcl[:, 0:1], [Rcl[:, 0:1].ap[0], [0, ND]])
                sca = work.tile([128, ND], fp32, tag="sca")
                nc.vector.tensor_tensor_scan(
                    out=sca[:, :], data0=Rb, data1=acoef[:, :],
                    initial=0.0, op0=OP.mult, op1=OP.add)
                scg = work.tile([128, ND], fp32, tag="scg")
                nc.vector.tensor_tensor_scan(
                    out=scg[:, :], data0=Rb, data1=gcoef[:, :],
                    initial=0.0, op0=OP.mult, op1=OP.add)
                sa = work.tile([128, 1], fp32, tag="sa")
                nc.vector.tensor_scalar(out=sa[:, :], in0=sca[:, ND - 1:ND],
                                        scalar1=float(lam_pow), scalar2=None,
                                        op0=OP.mult)
                u2 = work.tile([128, FJ], fp32, tag="u2")
                nc.vector.tensor_scalar(out=u2[:, :], in0=hb64[:, :],
                                        scalar1=scg[:, ND - 1:ND], scalar2=None,
                                        op0=OP.mult)
                nc.vector.scalar_tensor_tensor(
                    out=h1[:, :], in0=Ah[:, :], scalar=sa[:, 0:1],
                    in1=u2[:, :], op0=OP.mult, op1=OP.add)
                do_layernorm()

            def retrieval_step(rank, lam_pow):
                Ah = apply_A_to_Ah(rank)
                nc.vector.scalar_tensor_tensor(
                    out=h1[:, :], in0=Ah[:, :], scalar=float(lam_pow),
                    in1=hb64[:, :], op0=OP.mult, op1=OP.add)
                do_layernorm()

            def append(t):
                """hist[t] = h_s * sqrt(ETA * LAM^-t / (|h|^2+EPS))."""
                hn = ps_dn.tile([128, 1], fp32, tag="hn")
                nc.tensor.matmul(hn[:, :], bones[:, :], aahh[:, 1:2],
                                 start=True, stop=True)
                sd = work.tile([128, 1], fp32, tag="sd")
                nc.scalar.activation(out=sd[:, :], in_=hn[:, :], func=AF.Ln,
                                     bias=eps_t[:, 0:1], scale=1.0)
                iv = work.tile([128, 1], fp32, tag="iv")
                nc.scalar.activation(out=iv[:, :], in_=sd[:, :], func=AF.Exp,
                                     bias=0.0, scale=-0.5)
                nc.vector.tensor_scalar(out=hist[:, t, :], in0=hs64[:, :],
                                        scalar1=iv[:, 0:1],
                                        scalar2=float(math.sqrt(ETA * LAM ** (-t))),
                                        op0=OP.mult, op1=OP.mult)

            def h_base_step(t):
                """hb64 = (W_h h + Z_t) in 64p; hs64 = relu(hb64); hh prep."""
                for half in range(2):
                    hb_ps = ps_hb.tile([BL, 384], fp32, tag=f"hb{half}")
                    for ki in range(KI):
                        nc.tensor.matmul(hb_ps[:, :], hs128[:, :, ki],
                                         WhT[:, ki, half * 384:(half + 1) * 384],
                                         start=(ki == 0), stop=(ki == KI - 1))
                    if half == 0:
                        with nc.allow_low_precision("bf16 transpose staging"):
                            nc.scalar.copy(out=hbT[:, 0:384], in_=hb_ps[:, :])
                    else:
                        with nc.allow_low_precision("bf16 transpose staging"):
                            nc.vector.tensor_copy(out=hbT[:, 384:768], in_=hb_ps[:, :])
                htp = ps_tp.tile([128, KI, BL], bf16, tag="htp")
                for ki in range(KI):
                    nc.tensor.transpose(htp[:, ki, :],
                                        hbT[:, ki * 128:(ki + 1) * 128],
                                        idbf[0:BL, 0:BL])
                # remap e-layout transpose output into 64p + add Z:
                # hb64[64b+q, ph+2c] = htp[q + 64ph, c, b] + Z64[...]; the
                # b != ph blocks read a PE-swapped copy of htp.
                htps = work.tile([128, KI, BL], bf16, tag="htps")
                nc.vector.tensor_copy(out=htps[:, :, :], in_=htp[:, :, :])
                swp = ps_tp.tile([128, FJ], fp32, tag="swp")
                htps2d = mkap(htps[:, :, :], [htps[:, :, :].ap[0], [1, KI * BL]])
                swp2d = swp[:, :]
                nc.tensor.matmul(swp2d, swap64[:, :], htps2d,
                                 start=True, stop=True)
                for b in range(BL):
                    for ph in range(2):
                        if b == ph:
                            src = htp[64 * b:64 * (b + 1), :, b]
                        else:
                            sb_ = swp[64 * b:64 * (b + 1), b:FJ]
                            src = mkap(sb_, [sb_.ap[0], [sb_.ap[1][0] * 2, KI]])
                        hbo = hb64[64 * b:64 * (b + 1), ph:FJ]
                        hbo = mkap(hbo, [hbo.ap[0], [hbo.ap[1][0] * 2, KI]])
                        zo = Z64[64 * b:64 * (b + 1), t, ph:FJ]
                        zo = mkap(zo, [zo.ap[0], [zo.ap[1][0] * 2, KI]])
                        nc.vector.tensor_add(hbo, src, zo)
                relu_to_hs64(hb64[:, :])
                hh_prep()

            # ---------------- time loop ----------------
            # t = 0: h_base = Z_0 (A=0 -> h after 4 inners = relu(LN(Z_0)))
            nc.vector.tensor_copy(out=h1[:, :], in_=Z64[:, 0, :])
            do_layernorm()
            relu_to_hs64(ln64[:, :])
            make_hs128()
            hh_prep()
            append(0)

            for t in range(1, T - 1):
                h_base_step(t)
                for s in range(S_IN):
                    inner_step(t, LAM ** (t - 1))
                    relu_to_hs64(ln64[:, :])
                    if s == S_IN - 1:
                        make_hs128()
                    hh_prep()
                append(t)

            # final step: h_s = relu(LN(h_base + A h_s))
            h_base_step(T - 1)
            for s in range(S_IN):
                retrieval_step(RANK, LAM ** (RANK - 1))
                relu_to_hs64(ln64[:, :])
                if s == S_IN - 1:
                    make_hs128()

            # head + loss
            lb = work.tile([BL, 1], fp32, tag="lb")
            hs128f = persist.tile([128, BL, KI], fp32, tag="hs128f")
            nc.vector.tensor_copy(out=hs128f[:, :, :], in_=hs128[:, :, :])
            pred_ps = ps_hb.tile([BL, D_OUT], fp32, tag="hb0")
            for ki in range(KI):
                nc.tensor.matmul(pred_ps[:, :], hs128f[:, :, ki], hWT[:, ki, :],
                                 start=(ki == 0), stop=(ki == KI - 1))
            df = work.tile([BL, D_OUT], fp32, tag="df")
            nc.vector.tensor_sub(df[:, :], pred_ps[:, :], hc[:, :])
            df2 = work.tile([BL, D_OUT], fp32, tag="df2")
            se = work.tile([BL, 1], fp32, tag="se")
            nc.vector.tensor_mul(df2[:, :], df[:, :], df[:, :])
            nc.vector.tensor_reduce(out=se[:, :], in_=df2[:, :],
                                    axis=AX.X, op=OP.add)
            nc.vector.tensor_mul(se[:, :], se[:, :], nrm[:, :])
            nc.scalar.activation(out=lb[:, :], in_=se[:, :], func=AF.Ln,
                                 bias=1.0, scale=1.0)
            nc.sync.dma_start(out=loss_d[:, :], in_=lb[:, :])

    nc.compile()
    return nc


def _get_nc():
    if "nc" not in _CACHE:
        _CACHE["nc"] = _build()
    return _CACHE["nc"]


def _numpy_kernel(z_seq, clean_vec, W_h, W_g, b_h, alpha_fw, ln_gamma, ln_beta,
                  head_W, head_b):
    def _layernorm(x, g, b):
        mu = np.mean(x, axis=-1, keepdims=True)
        var = np.mean((x - mu) ** 2, axis=-1, keepdims=True)
        return g * (x - mu) / np.sqrt(var + LN_EPS) + b

    k = _compute_k(np.asarray(alpha_fw).reshape(()))
    h = np.zeros((B, D_H), np.float32)
    hist = np.zeros((B, T - 1, D_H), np.float32)
    coef = np.zeros((B, T - 1), np.float32)
    rank = 0
    W_hT = W_h.T.copy()
    Z = (z_seq.reshape(T * B, D_G) @ W_g.T).reshape(T, B, D_H) + b_h

    def apply_A(x):
        if rank == 0:
            return np.zeros_like(x)
        Hr = hist[:, :rank, :]
        proj = np.matmul(Hr, x[:, :, None])[:, :, 0]
        return np.matmul((coef[:, :rank] * proj)[:, None, :], Hr)[:, 0, :]

    for t in range(T - 1):
        h_base = h @ W_hT + Z[t]
        h_s = np.maximum(h_base, 0.0)
        for _ in range(S_IN):
            Ah = apply_A(h_s)
            dot = np.sum(h_s * Ah, axis=1, keepdims=True)
            n1 = np.linalg.norm(h_s, axis=1, keepdims=True) + 1e-6
            n2 = np.linalg.norm(Ah, axis=1, keepdims=True) + 1e-6
            R_pos = np.clip(dot / (n1 * n2 + 1e-6), 0.0, 1.0)
            a = 1.0 - (1.0 - R_pos) ** k
            h_s = (1.0 - a ** 2) * h_base + a * Ah
            h_s = np.maximum(_layernorm(h_s, ln_gamma, ln_beta), 0.0)
        h = h_s
        hn2 = np.sum(h * h, axis=1) + EPS
        coef[:, :rank] *= LAM
        coef[:, rank] = ETA / hn2
        hist[:, rank, :] = h
        rank += 1

    h_base = h @ W_hT + Z[T - 1]
    h_s = np.maximum(h_base, 0.0)
    for _ in range(S_IN):
        h_s = np.maximum(_layernorm(h_base + apply_A(h_s), ln_gamma, ln_beta), 0.0)

    pred = h_s @ head_W.T + head_b
    diff = pred - clean_vec
    per_sample_se = np.sum(diff ** 2, axis=1)
    norm_clean = np.sum(clean_vec ** 2, axis=1) + 1e-6
    rel_err = per_sample_se / norm_clean
    return np.asarray(np.mean(np.log1p(rel_err)), np.float32)


def _make_in_maps(z_seq, clean_vec, W_h, W_g, b_h, acoef, gcoef,
                  ln_gamma, ln_beta, head_W, head_b):
    in_maps = []
    for c in range(NCORES):
        sl = slice(c * BL, (c + 1) * BL)
        in_maps.append({
            "z": np.ascontiguousarray(
                z_seq[:, sl, :].reshape(T * BL, D_G), np.float32),
            "clean": np.ascontiguousarray(clean_vec[sl], np.float32),
            "W_h": np.asarray(W_h, np.float32),
            "W_g": np.asarray(W_g, np.float32),
            "b_h": np.asarray(b_h, np.float32),
            "ln_gamma": np.asarray(ln_gamma, np.float32),
            "ln_beta": np.asarray(ln_beta, np.float32),
            "head_W": np.asarray(head_W, np.float32),
            "head_b": np.asarray(head_b, np.float32),
            "acoef": acoef,
            "gcoef": gcoef,
        })
    return in_maps


def run_on_hw(inputs, trace=False, **kw):
    """Build + run on the 8 NeuronCores. Returns (loss, BassKernelResults)."""
    from concourse.bass_utils import run_bass_kernel_spmd
    k = _compute_k(np.asarray(inputs["alpha_fw"]).reshape(()))
    acoef, gcoef, fit_err = _fit_coefs(float(k))
    if fit_err > 2e-3:
        raise ValueError(f"polynomial fit too inaccurate: {fit_err}")
    nc = _get_nc()
    in_maps = _make_in_maps(
        inputs["z_seq"], inputs["clean_vec"], inputs["W_h"], inputs["W_g"],
        inputs["b_h"], acoef, gcoef, inputs["ln_gamma"], inputs["ln_beta"],
        inputs["head_W"], inputs["head_b"])
    res = run_bass_kernel_spmd(nc, in_maps, list(range(NCORES)),
                               trace=trace, **kw)
    losses = np.concatenate(
        [np.asarray(r["loss"]).reshape(-1) for r in res.results])
    return np.asarray(np.mean(losses), np.float32), res


def kernel(z_seq, clean_vec, W_h, W_g, b_h, alpha_fw, ln_gamma, ln_beta,
           head_W, head_b):
    inputs = dict(z_seq=z_seq, clean_vec=clean_vec, W_h=W_h, W_g=W_g, b_h=b_h,
                  alpha_fw=alpha_fw, ln_gamma=ln_gamma, ln_beta=ln_beta,
                  head_W=head_W, head_b=head_b)
    try:
        loss, _ = run_on_hw(inputs)
        return loss
    except Exception:
        import os
        if os.environ.get("K_NOFALLBACK"):
            raise
        return _numpy_kernel(
            np.asarray(z_seq, np.float32), np.asarray(clean_vec, np.float32),
            np.asarray(W_h, np.float32), np.asarray(W_g, np.float32),
            np.asarray(b_h, np.float32), alpha_fw,
            np.asarray(ln_gamma, np.float32), np.asarray(ln_beta, np.float32),
            np.asarray(head_W, np.float32), np.asarray(head_b, np.float32))
